# revision 7
# baseline (speedup 1.0000x reference)
"""Multi-head attention (B=2, S=4096, D=1024, H=16) on 8 TRN2 NeuronCores.

Sharding: data-parallel over batch (2) x tensor-parallel over head groups
(4 groups of 4 heads).  Core c handles batch c//4, head group c%4.
Each core computes its 4 heads' Q/K/V projections, attention, and a
partial output projection; the host sums the 4 partials per batch and
adds the output bias.

Device algorithm (per core, bf16 matmuls with fp32 PSUM accumulation):
  - Q^T, K^T computed feature-major ([d_k, S]); K^T zero-padded so the
    scores matmul contracts over 128 partitions.
  - V computed token-major with an appended ones column per head.
  - scores tile S^T[kv,q] = (K Q^T) in PSUM -> ScalarE exp(x/8) -> bf16.
    No max subtraction: |S/8| <= ~15 for this problem, exp stays finite.
  - O_aug^T[65,q] += [V|1]^T @ exp(S^T): row 64 accumulates the softmax
    denominator for free.  Normalize with VectorE reciprocal +
    GPSIMD partition-broadcast, then the output projection.
"""

import sys

if "/opt/trn_rl_repo" not in sys.path:
    sys.path.insert(0, "/opt/trn_rl_repo")

import hashlib
import os
import shutil

import ml_dtypes
import numpy as np

import concourse.bass as bass
import concourse.tile as tile
from concourse import bacc, bass2jax, mybir
from concourse.bass_utils import run_bass_kernel_spmd

# Memoize the (slow, deterministic) BIR->NEFF compile on disk so repeated
# kernel() invocations across processes skip the multi-minute walrus compile.
_NEFF_CACHE_DIR = os.path.expanduser("~/.cache/bass_neff_cache")
_orig_compile_bir_kernel = bass2jax.compile_bir_kernel


def _cached_compile_bir_kernel(bir_json, tmpdir, neff_name="file.neff"):
    try:
        os.makedirs(_NEFF_CACHE_DIR, exist_ok=True)
        key = hashlib.sha256(bir_json).hexdigest()
        cpath = os.path.join(_NEFF_CACHE_DIR, f"{key}_{neff_name}")
        dst = os.path.join(tmpdir, neff_name)
        if os.path.exists(cpath):
            shutil.copy(cpath, dst)
            return dst
        path = _orig_compile_bir_kernel(bir_json, tmpdir, neff_name)
        shutil.copy(path, cpath)
        return path
    except OSError:
        return _orig_compile_bir_kernel(bir_json, tmpdir, neff_name)


bass2jax.compile_bir_kernel = _cached_compile_bir_kernel

BF16 = mybir.dt.bfloat16
F32 = mybir.dt.float32
NPBF = ml_dtypes.bfloat16

B, S, D = 2, 4096, 1024
N_HEADS = 16
D_K = 64
HG = 4                  # head groups (one per tensor-parallel rank)
HPG = N_HEADS // HG     # heads per group = 4
FG = HPG * D_K          # features per group = 256
P = 128
SCALE = 0.125           # 1/sqrt(d_k)
SW = 512                # s-window for projections
QW = 1024               # q-window for attention
N_CORES = 8


def build_nc():
    nc = bacc.Bacc("TRN2", target_bir_lowering=False, debug=False,
                   num_devices=N_CORES)

    xqT = nc.dram_tensor("xqT", [P, D // P, S], BF16, kind="ExternalInput").ap()
    xkT = nc.dram_tensor("xkT", [P, D // P, S], BF16, kind="ExternalInput").ap()
    xvT = nc.dram_tensor("xvT", [P, D // P, S], BF16, kind="ExternalInput").ap()
    wqT = nc.dram_tensor("wqT", [P, D // P, FG], BF16, kind="ExternalInput").ap()
    wkT = nc.dram_tensor("wkT", [P, D // P, FG], BF16, kind="ExternalInput").ap()
    wvT = nc.dram_tensor("wvT", [P, D // P, FG], BF16, kind="ExternalInput").ap()
    woT = nc.dram_tensor("woT", [P, FG // P, D], BF16, kind="ExternalInput").ap()
    bqd = nc.dram_tensor("bq2", [P, FG // P], F32, kind="ExternalInput").ap()
    bkd = nc.dram_tensor("bk2", [P, FG // P], F32, kind="ExternalInput").ap()
    bvd = nc.dram_tensor("bv1", [1, FG], F32, kind="ExternalInput").ap()
    y = nc.dram_tensor("y", [S, D], F32, kind="ExternalOutput").ap()
    y_r = y.rearrange("(t p) o -> t p o", p=P)

    from contextlib import ExitStack
    with tile.TileContext(nc) as tc, ExitStack() as ctx:
        consts = ctx.enter_context(tc.tile_pool(name="consts", bufs=1))
        resident = ctx.enter_context(tc.tile_pool(name="resident", bufs=1))
        xw = ctx.enter_context(tc.tile_pool(name="xw", bufs=3))
        projp = ctx.enter_context(tc.tile_pool(name="projp", bufs=2, space="PSUM"))
        sS = ctx.enter_context(tc.tile_pool(name="sS", bufs=2, space="PSUM"))
        oP = ctx.enter_context(tc.tile_pool(name="oP", bufs=2, space="PSUM"))
        sE = ctx.enter_context(tc.tile_pool(name="sE", bufs=3))
        misc = ctx.enter_context(tc.tile_pool(name="misc", bufs=4))
        yout = ctx.enter_context(tc.tile_pool(name="yout", bufs=3))

        # ---- constants ----
        wq_sb = consts.tile([P, D // P, FG], BF16)
        nc.sync.dma_start(wq_sb, wqT)
        wk_sb = consts.tile([P, D // P, FG], BF16)
        nc.sync.dma_start(wk_sb, wkT)
        wv_sb = consts.tile([P, D // P, FG], BF16)
        nc.sync.dma_start(wv_sb, wvT)
        wo_sb = consts.tile([P, FG // P, D], BF16)
        nc.sync.dma_start(wo_sb, woT)
        bq_sb = consts.tile([P, FG // P], F32)
        nc.sync.dma_start(bq_sb, bqd)
        bk_sb = consts.tile([P, FG // P], F32)
        nc.sync.dma_start(bk_sb, bkd)
        bv_sb = consts.tile([P, FG], F32)
        nc.gpsimd.dma_start(out=bv_sb, in_=bvd.to_broadcast((P, FG)))

        # ---- resident intermediates ----
        # Q^T compact: partition = feature%128 (head pair), [P, 2, S]
        QT = resident.tile([P, FG // P, S], BF16)
        # K^T padded per head: [P, 4, S]; head hh occupies partitions
        # (hh%2)*64..+64 of plane hh, rest is zero.
        KTp = resident.tile([P, HPG, S], BF16)
        # V token-major + ones column: [P, 32, 4, 65]
        VA = resident.tile([P, S // P, HPG, D_K + 1], BF16)
        # O^T feature-major (normalized attention output)
        OT = resident.tile([P, FG // P, S], BF16)

        nc.vector.memset(KTp, 0.0)
        for hh in range(HPG):
            nc.vector.memset(VA[:, :, hh, D_K:D_K + 1], 1.0)

        # ---- projections ----
        def proj_windows(src, emit):
            for sw in range(S // SW):
                xt = xw.tile([P, D // P, SW], BF16, tag="xwin")
                nc.sync.dma_start(xt, src[:, :, sw * SW:(sw + 1) * SW])
                emit(sw, xt)

        def emit_k(sw, xt):
            for fo in range(FG // P):
                ps = projp.tile([P, SW], F32, tag="pp")
                for ko in range(D // P):
                    nc.tensor.matmul(ps, lhsT=wk_sb[:, ko, fo * P:(fo + 1) * P],
                                     rhs=xt[:, ko, :],
                                     start=(ko == 0), stop=(ko == D // P - 1))
                sl = slice(sw * SW, (sw + 1) * SW)
                for half in range(2):
                    hh = fo * 2 + half
                    nc.vector.tensor_tensor(
                        KTp[half * D_K:(half + 1) * D_K, hh, sl],
                        ps[half * D_K:(half + 1) * D_K, :],
                        bk_sb[half * D_K:(half + 1) * D_K, fo:fo + 1]
                        .to_broadcast((D_K, SW)),
                        mybir.AluOpType.add)

        def emit_v(sw, xt):
            for st in range(SW // P):
                ps = projp.tile([P, FG], F32, tag="pp")
                for ko in range(D // P):
                    nc.tensor.matmul(ps, lhsT=xt[:, ko, st * P:(st + 1) * P],
                                     rhs=wv_sb[:, ko, :],
                                     start=(ko == 0), stop=(ko == D // P - 1))
                for hh in range(HPG):
                    nc.vector.tensor_tensor(
                        VA[:, sw * (SW // P) + st, hh, 0:D_K],
                        ps[:, hh * D_K:(hh + 1) * D_K],
                        bv_sb[:, hh * D_K:(hh + 1) * D_K],
                        mybir.AluOpType.add)

        def emit_q(sw, xt):
            for fo in range(FG // P):
                ps = projp.tile([P, SW], F32, tag="pp")
                for ko in range(D // P):
                    nc.tensor.matmul(ps, lhsT=wq_sb[:, ko, fo * P:(fo + 1) * P],
                                     rhs=xt[:, ko, :],
                                     start=(ko == 0), stop=(ko == D // P - 1))
                nc.vector.tensor_tensor(
                    QT[:, fo, sw * SW:(sw + 1) * SW], ps,
                    bq_sb[:, fo:fo + 1].to_broadcast((P, SW)),
                    mybir.AluOpType.add)

        proj_windows(xkT, emit_k)
        proj_windows(xvT, emit_v)
        proj_windows(xqT, emit_q)

        # ---- attention + output projection ----
        KV = S // P  # 32 kv tiles
        for qw in range(S // QW):
            for hh in range(HPG):
                oacc = []
                for _c in range(QW // SW):
                    ot_acc = oP.tile([D_K + 1, SW], F32, tag="oacc",
                                     name=f"oacc_{qw}_{hh}_{_c}")
                    oacc.append(ot_acc)
                for kv in range(KV):
                    st = sS.tile([P, QW], F32, tag="st")
                    for c in range(QW // SW):
                        nc.tensor.matmul(
                            st[:, c * SW:(c + 1) * SW],
                            lhsT=KTp[:, hh, kv * P:(kv + 1) * P],
                            rhs=QT[:, hh // 2, qw * QW + c * SW: qw * QW + (c + 1) * SW],
                            start=True, stop=True)
                    et = sE.tile([P, QW], BF16, tag="et")
                    nc.scalar.activation(et, st,
                                         mybir.ActivationFunctionType.Exp,
                                         scale=SCALE)
                    for c in range(QW // SW):
                        nc.tensor.matmul(
                            oacc[c], lhsT=VA[:, kv, hh, :],
                            rhs=et[:, c * SW:(c + 1) * SW],
                            start=(kv == 0), stop=(kv == KV - 1))
                # normalize: divide rows 0..63 by denominator row 64
                for c in range(QW // SW):
                    rc = misc.tile([1, SW], F32, tag="rc")
                    nc.vector.reciprocal(rc, oacc[c][D_K:D_K + 1, :])
                    rcb = misc.tile([D_K, SW], F32, tag="rcb")
                    nc.gpsimd.partition_broadcast(rcb, rc)
                    nc.vector.tensor_tensor(
                        OT[(hh % 2) * D_K:(hh % 2 + 1) * D_K, hh // 2,
                           qw * QW + c * SW: qw * QW + (c + 1) * SW],
                        oacc[c][0:D_K, :], rcb, mybir.AluOpType.mult)

            # output projection for this q window
            for qt in range(qw * (QW // P), (qw + 1) * (QW // P)):
                for on in range(D // SW):
                    yp = projp.tile([P, SW], F32, tag="pp")
                    for fo in range(FG // P):
                        nc.tensor.matmul(
                            yp, lhsT=OT[:, fo, qt * P:(qt + 1) * P],
                            rhs=wo_sb[:, fo, on * SW:(on + 1) * SW],
                            start=(fo == 0), stop=(fo == FG // P - 1))
                    ys = yout.tile([P, SW], F32, tag="ys")
                    nc.vector.tensor_copy(ys, yp)
                    nc.sync.dma_start(y_r[qt, :, on * SW:(on + 1) * SW], ys)

    nc.compile()
    return nc


def _dmajor(a2d, free):
    """[rows, 1024-like dim] bf16 -> [128, dim//128, free] with partition =
    inner index of the leading (transposed) dim."""
    d = a2d.shape[1]
    t = np.ascontiguousarray(a2d.astype(NPBF).T)          # [d, free]
    return np.ascontiguousarray(t.reshape(d // P, P, free).transpose(1, 0, 2))


_NC_CACHE = {}


def make_in_maps(query, key, value, Wq, bq, Wk, bk, Wv, bv, Wo, bo):
    query = np.asarray(query, np.float32)
    key = np.asarray(key, np.float32)
    value = np.asarray(value, np.float32)

    # per-batch transposed bf16 activations (shared by 4 cores each)
    xT = {b: {"q": _dmajor(query[b], S), "k": _dmajor(key[b], S),
              "v": _dmajor(value[b], S)} for b in range(B)}

    in_maps = []
    for c in range(N_CORES):
        b, hg = divmod(c, HG)
        fsl = slice(hg * FG, (hg + 1) * FG)
        in_maps.append({
            "xqT": xT[b]["q"], "xkT": xT[b]["k"], "xvT": xT[b]["v"],
            "wqT": _dmajor(np.asarray(Wq)[fsl], FG),
            "wkT": _dmajor(np.asarray(Wk)[fsl], FG),
            "wvT": _dmajor(np.asarray(Wv)[fsl], FG),
            "woT": _dmajor(np.asarray(Wo)[:, fsl], D),
            "bq2": np.ascontiguousarray(
                np.asarray(bq, np.float32)[fsl].reshape(FG // P, P).T),
            "bk2": np.ascontiguousarray(
                np.asarray(bk, np.float32)[fsl].reshape(FG // P, P).T),
            "bv1": np.asarray(bv, np.float32)[fsl].reshape(1, FG),
        })
    return in_maps


def kernel(query, key, value, Wq, bq, Wk, bk, Wv, bv, Wo, bo):
    if "nc" not in _NC_CACHE:
        _NC_CACHE["nc"] = build_nc()
    nc = _NC_CACHE["nc"]

    in_maps = make_in_maps(query, key, value, Wq, bq, Wk, bk, Wv, bv, Wo, bo)
    res = run_bass_kernel_spmd(nc, in_maps, core_ids=list(range(N_CORES)))

    out = np.zeros((B, S, D), np.float32)
    for c in range(N_CORES):
        out[c // HG] += res.results[c]["y"]
    out += np.asarray(bo, np.float32)
    return out


# revision 31
# speedup vs baseline: 6.3563x; 6.3563x over previous
"""Multi-head attention (B=2, S=4096, D=1024, H=16) on 8 TRN2 NeuronCores.

Sharding: data-parallel over batch (2) x tensor-parallel over head groups
(4 groups of 4 heads).  Core c handles batch c//4, head group c%4.
Each core computes its 4 heads' Q/K/V projections, attention, and a
partial output projection; the host sums the 4 partials per batch and
adds the output bias.

Device algorithm (per core, bf16 matmuls with fp32 PSUM accumulation):
  - Q^T, K^T computed feature-major ([d_k, S]); K^T zero-padded so the
    scores matmul contracts over 128 partitions.
  - V computed token-major with an appended ones column per head.
  - scores tile S^T[kv,q] = (K Q^T) in PSUM -> ScalarE exp(x/8) -> bf16.
    No max subtraction: |S/8| <= ~15 for this problem, exp stays finite.
  - O_aug^T[65,q] += [V|1]^T @ exp(S^T): row 64 accumulates the softmax
    denominator for free.  Normalize with VectorE reciprocal +
    GPSIMD partition-broadcast, then the output projection.
"""

import sys

if "/opt/trn_rl_repo" not in sys.path:
    sys.path.insert(0, "/opt/trn_rl_repo")

import hashlib
import os
import shutil

import ml_dtypes
import numpy as np

import concourse.bass as bass
import concourse.tile as tile
from concourse import bacc, bass2jax, mybir
from concourse.bass_utils import run_bass_kernel_spmd

# Memoize the (slow, deterministic) BIR->NEFF compile on disk so repeated
# kernel() invocations across processes skip the multi-minute walrus compile.
_NEFF_CACHE_DIR = os.path.expanduser("~/.cache/bass_neff_cache")
_orig_compile_bir_kernel = bass2jax.compile_bir_kernel


def _cached_compile_bir_kernel(bir_json, tmpdir, neff_name="file.neff"):
    try:
        os.makedirs(_NEFF_CACHE_DIR, exist_ok=True)
        key = hashlib.sha256(bir_json).hexdigest()
        cpath = os.path.join(_NEFF_CACHE_DIR, f"{key}_{neff_name}")
        dst = os.path.join(tmpdir, neff_name)
        if os.path.exists(cpath):
            shutil.copy(cpath, dst)
            return dst
        path = _orig_compile_bir_kernel(bir_json, tmpdir, neff_name)
        shutil.copy(path, cpath)
        return path
    except OSError:
        return _orig_compile_bir_kernel(bir_json, tmpdir, neff_name)


bass2jax.compile_bir_kernel = _cached_compile_bir_kernel

BF16 = mybir.dt.bfloat16
F32 = mybir.dt.float32
NPBF = ml_dtypes.bfloat16

B, S, D = 2, 4096, 1024
N_HEADS = 16
D_K = 64
HG = 4                  # head groups (one per tensor-parallel rank)
HPG = N_HEADS // HG     # heads per group = 4
FG = HPG * D_K          # features per group = 256
P = 128
SCALE = 0.125           # 1/sqrt(d_k)
SW = 512                # s-window for projections
QW = 1024               # q-window for attention
N_CORES = 8


def build_nc(include_proj=True, include_attn=True, include_final=True,
             norm="decoupled", reps=1, swpipe=True, interleave01=True,
             spread=True):
    nc = bacc.Bacc("TRN2", target_bir_lowering=False, debug=False,
                   num_devices=N_CORES)

    xqT = nc.dram_tensor("xqT", [P, D // P, S], BF16, kind="ExternalInput").ap()
    xkT = nc.dram_tensor("xkT", [P, D // P, S], BF16, kind="ExternalInput").ap()
    xvT = nc.dram_tensor("xvT", [P, D // P, S], BF16, kind="ExternalInput").ap()
    wqT = nc.dram_tensor("wqT", [P, D // P, FG], BF16, kind="ExternalInput").ap()
    wkT = nc.dram_tensor("wkT", [P, D // P, FG], BF16, kind="ExternalInput").ap()
    wvT = nc.dram_tensor("wvT", [P, D // P, FG], BF16, kind="ExternalInput").ap()
    woT = nc.dram_tensor("woT", [P, FG // P, D], BF16, kind="ExternalInput").ap()
    bqd = nc.dram_tensor("bq2", [P, FG // P], F32, kind="ExternalInput").ap()
    bkd = nc.dram_tensor("bk2", [P, FG // P], F32, kind="ExternalInput").ap()
    bvd = nc.dram_tensor("bv1", [1, FG], F32, kind="ExternalInput").ap()
    y = nc.dram_tensor("y", [S, D], F32, kind="ExternalOutput").ap()
    y_r = y.rearrange("(t p) o -> t p o", p=P)

    from contextlib import ExitStack
    with tile.TileContext(nc) as tc, ExitStack() as ctx:
        consts = ctx.enter_context(tc.tile_pool(name="consts", bufs=1))
        resident = ctx.enter_context(tc.tile_pool(name="resident", bufs=1))
        xw = ctx.enter_context(tc.tile_pool(name="xw", bufs=4))
        projp = ctx.enter_context(tc.tile_pool(name="projp", bufs=2, space="PSUM"))
        sS = ctx.enter_context(tc.tile_pool(name="sS", bufs=2, space="PSUM"))
        oP = ctx.enter_context(tc.tile_pool(name="oP", bufs=2, space="PSUM"))
        sE = ctx.enter_context(tc.tile_pool(name="sE", bufs=4))
        misc = ctx.enter_context(tc.tile_pool(name="misc", bufs=6))
        yout = ctx.enter_context(tc.tile_pool(name="yout", bufs=4))

        # ---- constants ----
        wq_sb = consts.tile([P, D // P, FG], BF16)
        nc.sync.dma_start(wq_sb, wqT)
        wk_sb = consts.tile([P, D // P, FG], BF16)
        nc.sync.dma_start(wk_sb, wkT)
        wv_sb = consts.tile([P, D // P, FG], BF16)
        nc.sync.dma_start(wv_sb, wvT)
        wo_sb = consts.tile([P, FG // P, D], BF16)
        nc.sync.dma_start(wo_sb, woT)
        bq_sb = consts.tile([P, FG // P], F32)
        nc.sync.dma_start(bq_sb, bqd)
        bk_sb = consts.tile([P, FG // P], F32)
        nc.sync.dma_start(bk_sb, bkd)
        bv_sb = consts.tile([P, FG], F32)
        nc.gpsimd.dma_start(out=bv_sb, in_=bvd.to_broadcast((P, FG)))

        # ---- resident intermediates ----
        # Q^T compact: partition = feature%128 (head pair), [P, 2, S]
        QT = resident.tile([P, FG // P, S], BF16)
        # K^T padded per head: [P, 4, S]; head hh occupies partitions
        # (hh%2)*64..+64 of plane hh, rest is zero.
        KTp = resident.tile([P, HPG, S], BF16)
        # V token-major + ones column: [P, 32, 4, 65]
        VA = resident.tile([P, S // P, HPG, D_K + 1], BF16)
        # O^T feature-major (normalized attention output)
        OT = resident.tile([P, FG // P, S], BF16)

        nc.vector.memset(KTp, 0.0)
        for hh in range(HPG):
            nc.vector.memset(VA[:, :, hh, D_K:D_K + 1], 1.0)
        if not include_proj:
            nc.vector.memset(QT, 0.01)
            nc.vector.memset(VA[:, :, :, 0:D_K], 0.01)
            nc.vector.memset(KTp, 0.01)
        if not include_attn:
            nc.vector.memset(OT, 0.01)

        # ---- projections ----
        def proj_windows(src, emit, windows=None):
            for sw in windows if windows is not None else range(S // SW):
                xt = xw.tile([P, D // P, SW], BF16, tag="xwin")
                nc.sync.dma_start(xt, src[:, :, sw * SW:(sw + 1) * SW])
                emit(sw, xt)

        def emit_k(sw, xt):
            for fo in range(FG // P):
                ps = projp.tile([P, SW], F32, tag="pp")
                for ko in range(D // P):
                    nc.tensor.matmul(ps, lhsT=wk_sb[:, ko, fo * P:(fo + 1) * P],
                                     rhs=xt[:, ko, :],
                                     start=(ko == 0), stop=(ko == D // P - 1))
                sl = slice(sw * SW, (sw + 1) * SW)
                for half in range(2):
                    hh = fo * 2 + half
                    nc.vector.tensor_tensor(
                        KTp[half * D_K:(half + 1) * D_K, hh, sl],
                        ps[half * D_K:(half + 1) * D_K, :],
                        bk_sb[half * D_K:(half + 1) * D_K, fo:fo + 1]
                        .to_broadcast((D_K, SW)),
                        mybir.AluOpType.add)

        def emit_v(sw, xt):
            for st in range(SW // P):
                ps = projp.tile([P, FG], F32, tag="pp")
                for ko in range(D // P):
                    nc.tensor.matmul(ps, lhsT=xt[:, ko, st * P:(st + 1) * P],
                                     rhs=wv_sb[:, ko, :],
                                     start=(ko == 0), stop=(ko == D // P - 1))
                for hh in range(HPG):
                    nc.vector.tensor_tensor(
                        VA[:, sw * (SW // P) + st, hh, 0:D_K],
                        ps[:, hh * D_K:(hh + 1) * D_K],
                        bv_sb[:, hh * D_K:(hh + 1) * D_K],
                        mybir.AluOpType.add)

        def emit_q(sw, xt):
            for fo in range(FG // P):
                ps = projp.tile([P, SW], F32, tag="pp")
                for ko in range(D // P):
                    nc.tensor.matmul(ps, lhsT=wq_sb[:, ko, fo * P:(fo + 1) * P],
                                     rhs=xt[:, ko, :],
                                     start=(ko == 0), stop=(ko == D // P - 1))
                nc.vector.tensor_tensor(
                    QT[:, fo, sw * SW:(sw + 1) * SW], ps,
                    bq_sb[:, fo:fo + 1].to_broadcast((P, SW)),
                    mybir.AluOpType.add)

        KV = S // P  # 32 kv tiles

        def alloc_oacc(key):
            oacc = []
            for _c in range(QW // SW):
                ot_acc = oP.tile([D_K + 1, SW], F32, tag="oacc",
                                 name=f"oacc_{key}_{_c}")
                oacc.append(ot_acc)
            return oacc

        def _mm2(oacc, hh, et, kv):
            for c in range(QW // SW):
                nc.tensor.matmul(
                    oacc[c], lhsT=VA[:, kv, hh, :],
                    rhs=et[:, c * SW:(c + 1) * SW],
                    start=(kv == 0), stop=(kv == KV - 1))

        def emit_attn(qw, hh, oacc, kv_range, pend=None):
            # Software-pipelined PE stream: emit kv's scores matmuls before
            # the previous kv's PV matmuls so PE works while ScalarE computes
            # exp, instead of stalling behind the exp->mm2 dependency.
            for kv in kv_range:
                st = sS.tile([P, QW], F32, tag="st")
                for c in range(QW // SW):
                    nc.tensor.matmul(
                        st[:, c * SW:(c + 1) * SW],
                        lhsT=KTp[:, hh, kv * P:(kv + 1) * P],
                        rhs=QT[:, hh // 2, qw * QW + c * SW: qw * QW + (c + 1) * SW],
                        start=True, stop=True)
                et = sE.tile([P, QW], BF16, tag="et")
                nc.scalar.activation(et, st,
                                     mybir.ActivationFunctionType.Exp,
                                     scale=SCALE)
                if not swpipe:
                    _mm2(oacc, hh, et, kv)
                    continue
                if pend is not None:
                    _mm2(oacc, hh, *pend)
                pend = (et, kv)
            return pend

        def emit_norm(qw, hh, oacc):
            # normalize: divide rows 0..63 by denominator row 64.
            # Keep the gpsimd broadcast OFF the PSUM-release chain: copy the
            # unnormalized rows + reciprocal out first (DVE only), then
            # broadcast + multiply from SBUF.
            for c in range(QW // SW):
                ots = slice(qw * QW + c * SW, qw * QW + (c + 1) * SW)
                otdst = OT[(hh % 2) * D_K:(hh % 2 + 1) * D_K, hh // 2, ots]
                if norm == "none":
                    nc.vector.tensor_copy(otdst, oacc[c][0:D_K, :])
                    continue
                rc = misc.tile([1, SW], F32, tag="rc")
                nc.vector.reciprocal(rc, oacc[c][D_K:D_K + 1, :])
                if norm == "decoupled":
                    otu = misc.tile([D_K, SW], F32, tag="otu")
                    nc.vector.tensor_copy(otu, oacc[c][0:D_K, :])
                    rcb = misc.tile([D_K, SW], F32, tag="rcb")
                    nc.gpsimd.partition_broadcast(rcb, rc)
                    nc.vector.tensor_tensor(otdst, otu, rcb,
                                            mybir.AluOpType.mult)
                else:  # "chained": original form
                    rcb = misc.tile([D_K, SW], F32, tag="rcb")
                    nc.gpsimd.partition_broadcast(rcb, rc)
                    nc.vector.tensor_tensor(otdst, oacc[c][0:D_K, :], rcb,
                                            mybir.AluOpType.mult)

        for _rep in range(reps):
          if include_proj:
            # Q for the first attention q-window, then K/V windows
            # interleaved with head 0's attention kv-chunks so ScalarE (exp)
            # starts working during the projection phase.
            proj_windows(xqT, emit_q, windows=range(QW // SW))
            if include_attn:
                if interleave01:
                    oacc00 = alloc_oacc(f"h0_{_rep}")
                    pend = None
                    for w in range(S // SW):
                        proj_windows(xkT, emit_k, windows=[w])
                        proj_windows(xvT, emit_v, windows=[w])
                        pend = emit_attn(0, 0, oacc00,
                                         range(w * (SW // P),
                                               (w + 1) * (SW // P)),
                                         pend)
                    if pend is not None:
                        _mm2(oacc00, 0, *pend)
                    emit_norm(0, 0, oacc00)
                else:
                    proj_windows(xkT, emit_k)
                    proj_windows(xvT, emit_v)
                    oacc00 = alloc_oacc(f"h0_{_rep}")
                    pend = emit_attn(0, 0, oacc00, range(KV))
                    if pend is not None:
                        _mm2(oacc00, 0, *pend)
                    emit_norm(0, 0, oacc00)
            else:
                proj_windows(xkT, emit_k)
                proj_windows(xvT, emit_v)
          elif include_attn:
            oacc00 = alloc_oacc(f"h0_{_rep}")
            pend = emit_attn(0, 0, oacc00, range(KV))
            if pend is not None:
                _mm2(oacc00, 0, *pend)
            emit_norm(0, 0, oacc00)

          # ---- attention + output projection ----
          def emit_final(qt):
              for on in range(D // SW):
                  yp = projp.tile([P, SW], F32, tag="pp")
                  for fo in range(FG // P):
                      nc.tensor.matmul(
                          yp, lhsT=OT[:, fo, qt * P:(qt + 1) * P],
                          rhs=wo_sb[:, fo, on * SW:(on + 1) * SW],
                          start=(fo == 0), stop=(fo == FG // P - 1))
                  ys = yout.tile([P, SW], F32, tag="ys")
                  nc.vector.tensor_copy(ys, yp)
                  nc.sync.dma_start(y_r[qt, :, on * SW:(on + 1) * SW], ys)

          QT_PER_W = QW // P  # q-tiles per window
          for qw in range(S // QW):
            for hh in range(HPG if include_attn else 0):
                if not (qw == 0 and hh == 0):
                    oacc = alloc_oacc(f"{_rep}_{qw}_{hh}")
                    pend = emit_attn(qw, hh, oacc, range(KV))
                    if pend is not None:
                        _mm2(oacc, hh, *pend)
                    emit_norm(qw, hh, oacc)
                # spread the previous window's output projection and the
                # next window's Q projection across this window's heads so
                # PE-side filler work never starves ScalarE for long.
                if spread:
                    if include_final and qw > 0:
                        qt0 = (qw - 1) * QT_PER_W
                        emit_final(qt0 + 2 * hh)
                        emit_final(qt0 + 2 * hh + 1)
                    if include_proj and qw + 1 < S // QW and hh in (1, 3):
                        proj_windows(xqT, emit_q,
                                     windows=[(qw + 1) * (QW // SW) + hh // 2])
            if include_attn and not spread:
                if include_proj and qw + 1 < S // QW:
                    lo = (qw + 1) * (QW // SW)
                    proj_windows(xqT, emit_q, windows=range(lo, lo + QW // SW))
                if include_final:
                    for qt in range(qw * QT_PER_W, (qw + 1) * QT_PER_W):
                        emit_final(qt)
            if not include_attn:
                if include_proj and qw + 1 < S // QW:
                    lo = (qw + 1) * (QW // SW)
                    proj_windows(xqT, emit_q, windows=range(lo, lo + QW // SW))
                if include_final and qw > 0:
                    for qt in range((qw - 1) * QT_PER_W, qw * QT_PER_W):
                        emit_final(qt)
          if include_final and (spread or not include_attn):
            for qt in range((S // QW - 1) * QT_PER_W, S // P):
                emit_final(qt)

    nc.compile()
    return nc


def _dmajor(a2d, free):
    """[rows, 1024-like dim] bf16 -> [128, dim//128, free] with partition =
    inner index of the leading (transposed) dim."""
    d = a2d.shape[1]
    t = np.ascontiguousarray(a2d.astype(NPBF).T)          # [d, free]
    return np.ascontiguousarray(t.reshape(d // P, P, free).transpose(1, 0, 2))


_NC_CACHE = {}


def make_in_maps(query, key, value, Wq, bq, Wk, bk, Wv, bv, Wo, bo):
    query = np.asarray(query, np.float32)
    key = np.asarray(key, np.float32)
    value = np.asarray(value, np.float32)

    # per-batch transposed bf16 activations (shared by 4 cores each)
    xT = {b: {"q": _dmajor(query[b], S), "k": _dmajor(key[b], S),
              "v": _dmajor(value[b], S)} for b in range(B)}

    in_maps = []
    for c in range(N_CORES):
        b, hg = divmod(c, HG)
        fsl = slice(hg * FG, (hg + 1) * FG)
        in_maps.append({
            "xqT": xT[b]["q"], "xkT": xT[b]["k"], "xvT": xT[b]["v"],
            "wqT": _dmajor(np.asarray(Wq)[fsl], FG),
            "wkT": _dmajor(np.asarray(Wk)[fsl], FG),
            "wvT": _dmajor(np.asarray(Wv)[fsl], FG),
            "woT": _dmajor(np.asarray(Wo)[:, fsl], D),
            "bq2": np.ascontiguousarray(
                np.asarray(bq, np.float32)[fsl].reshape(FG // P, P).T),
            "bk2": np.ascontiguousarray(
                np.asarray(bk, np.float32)[fsl].reshape(FG // P, P).T),
            "bv1": np.asarray(bv, np.float32)[fsl].reshape(1, FG),
        })
    return in_maps


def kernel(query, key, value, Wq, bq, Wk, bk, Wv, bv, Wo, bo):
    if "nc" not in _NC_CACHE:
        _NC_CACHE["nc"] = build_nc()
    nc = _NC_CACHE["nc"]

    in_maps = make_in_maps(query, key, value, Wq, bq, Wk, bk, Wv, bv, Wo, bo)
    res = run_bass_kernel_spmd(nc, in_maps, core_ids=list(range(N_CORES)))

    out = np.zeros((B, S, D), np.float32)
    for c in range(N_CORES):
        out[c // HG] += res.results[c]["y"]
    out += np.asarray(bo, np.float32)
    return out


# revision 33
# speedup vs baseline: 6.5732x; 1.0341x over previous
"""Multi-head attention (B=2, S=4096, D=1024, H=16) on 8 TRN2 NeuronCores.

Sharding: data-parallel over batch (2) x tensor-parallel over head groups
(4 groups of 4 heads).  Core c handles batch c//4, head group c%4.
Each core computes its 4 heads' Q/K/V projections, attention, and a
partial output projection; the host sums the 4 partials per batch and
adds the output bias.

Device algorithm (per core, bf16 matmuls with fp32 PSUM accumulation):
  - Q^T, K^T computed feature-major ([d_k, S]); K^T zero-padded so the
    scores matmul contracts over 128 partitions.
  - V computed token-major with an appended ones column per head.
  - scores tile S^T[kv,q] = (K Q^T) in PSUM -> ScalarE exp(x/8) -> bf16.
    No max subtraction: |S/8| <= ~15 for this problem, exp stays finite.
  - O_aug^T[65,q] += [V|1]^T @ exp(S^T): row 64 accumulates the softmax
    denominator for free.  Normalize with VectorE reciprocal +
    GPSIMD partition-broadcast, then the output projection.
"""

import sys

if "/opt/trn_rl_repo" not in sys.path:
    sys.path.insert(0, "/opt/trn_rl_repo")

import hashlib
import os
import shutil

import ml_dtypes
import numpy as np

import concourse.bass as bass
import concourse.tile as tile
from concourse import bacc, bass2jax, mybir
from concourse.bass_utils import run_bass_kernel_spmd

# Memoize the (slow, deterministic) BIR->NEFF compile on disk so repeated
# kernel() invocations across processes skip the multi-minute walrus compile.
_NEFF_CACHE_DIR = os.path.expanduser("~/.cache/bass_neff_cache")
_orig_compile_bir_kernel = bass2jax.compile_bir_kernel


def _cached_compile_bir_kernel(bir_json, tmpdir, neff_name="file.neff"):
    try:
        os.makedirs(_NEFF_CACHE_DIR, exist_ok=True)
        key = hashlib.sha256(bir_json).hexdigest()
        cpath = os.path.join(_NEFF_CACHE_DIR, f"{key}_{neff_name}")
        dst = os.path.join(tmpdir, neff_name)
        if os.path.exists(cpath):
            shutil.copy(cpath, dst)
            return dst
        path = _orig_compile_bir_kernel(bir_json, tmpdir, neff_name)
        shutil.copy(path, cpath)
        return path
    except OSError:
        return _orig_compile_bir_kernel(bir_json, tmpdir, neff_name)


bass2jax.compile_bir_kernel = _cached_compile_bir_kernel

BF16 = mybir.dt.bfloat16
F32 = mybir.dt.float32
NPBF = ml_dtypes.bfloat16

B, S, D = 2, 4096, 1024
N_HEADS = 16
D_K = 64
HG = 4                  # head groups (one per tensor-parallel rank)
HPG = N_HEADS // HG     # heads per group = 4
FG = HPG * D_K          # features per group = 256
P = 128
SCALE = 0.125           # 1/sqrt(d_k)
SW = 512                # s-window for projections
QW = 1024               # q-window for attention
N_CORES = 8


def build_nc(include_proj=True, include_attn=True, include_final=True,
             norm="decoupled", reps=1, swpipe=True, interleave01=True,
             spread=True, deep=False):
    nc = bacc.Bacc("TRN2", target_bir_lowering=False, debug=False,
                   num_devices=N_CORES)

    xqT = nc.dram_tensor("xqT", [P, D // P, S], BF16, kind="ExternalInput").ap()
    xkT = nc.dram_tensor("xkT", [P, D // P, S], BF16, kind="ExternalInput").ap()
    xvT = nc.dram_tensor("xvT", [P, D // P, S], BF16, kind="ExternalInput").ap()
    wqT = nc.dram_tensor("wqT", [P, D // P, FG], BF16, kind="ExternalInput").ap()
    wkT = nc.dram_tensor("wkT", [P, D // P, FG], BF16, kind="ExternalInput").ap()
    wvT = nc.dram_tensor("wvT", [P, D // P, FG], BF16, kind="ExternalInput").ap()
    woT = nc.dram_tensor("woT", [P, FG // P, D], BF16, kind="ExternalInput").ap()
    bqd = nc.dram_tensor("bq2", [P, FG // P], F32, kind="ExternalInput").ap()
    bkd = nc.dram_tensor("bk2", [P, FG // P], F32, kind="ExternalInput").ap()
    bvd = nc.dram_tensor("bv1", [1, FG], F32, kind="ExternalInput").ap()
    y = nc.dram_tensor("y", [S, D], F32, kind="ExternalOutput").ap()
    y_r = y.rearrange("(t p) o -> t p o", p=P)

    from contextlib import ExitStack
    with tile.TileContext(nc) as tc, ExitStack() as ctx:
        consts = ctx.enter_context(tc.tile_pool(name="consts", bufs=1))
        resident = ctx.enter_context(tc.tile_pool(name="resident", bufs=1))
        xw = ctx.enter_context(tc.tile_pool(name="xw", bufs=4))
        projp = ctx.enter_context(tc.tile_pool(name="projp", bufs=2, space="PSUM"))
        sS = ctx.enter_context(tc.tile_pool(name="sS", bufs=2, space="PSUM"))
        oP = ctx.enter_context(tc.tile_pool(name="oP", bufs=2, space="PSUM"))
        sE = ctx.enter_context(tc.tile_pool(name="sE", bufs=6 if deep else 4))
        misc = ctx.enter_context(tc.tile_pool(name="misc", bufs=8 if deep else 6))
        yout = ctx.enter_context(tc.tile_pool(name="yout", bufs=4))

        # ---- constants ----
        wq_sb = consts.tile([P, D // P, FG], BF16)
        nc.sync.dma_start(wq_sb, wqT)
        wk_sb = consts.tile([P, D // P, FG], BF16)
        nc.sync.dma_start(wk_sb, wkT)
        wv_sb = consts.tile([P, D // P, FG], BF16)
        nc.sync.dma_start(wv_sb, wvT)
        wo_sb = consts.tile([P, FG // P, D], BF16)
        nc.sync.dma_start(wo_sb, woT)
        bq_sb = consts.tile([P, FG // P], F32)
        nc.sync.dma_start(bq_sb, bqd)
        bk_sb = consts.tile([P, FG // P], F32)
        nc.sync.dma_start(bk_sb, bkd)
        bv_sb = consts.tile([P, FG], F32)
        nc.gpsimd.dma_start(out=bv_sb, in_=bvd.to_broadcast((P, FG)))

        # ---- resident intermediates ----
        # Q^T compact: partition = feature%128 (head pair), [P, 2, S]
        QT = resident.tile([P, FG // P, S], BF16)
        # K^T padded per head: [P, 4, S]; head hh occupies partitions
        # (hh%2)*64..+64 of plane hh, rest is zero.
        KTp = resident.tile([P, HPG, S], BF16)
        # V token-major + ones column: [P, 32, 4, 65]
        VA = resident.tile([P, S // P, HPG, D_K + 1], BF16)
        # O^T feature-major (normalized attention output)
        OT = resident.tile([P, FG // P, S], BF16)

        nc.vector.memset(KTp, 0.0)
        for hh in range(HPG):
            nc.vector.memset(VA[:, :, hh, D_K:D_K + 1], 1.0)
        if not include_proj:
            nc.vector.memset(QT, 0.01)
            nc.vector.memset(VA[:, :, :, 0:D_K], 0.01)
            nc.vector.memset(KTp, 0.01)
        if not include_attn:
            nc.vector.memset(OT, 0.01)

        # ---- projections ----
        def proj_windows(src, emit, windows=None):
            for sw in windows if windows is not None else range(S // SW):
                xt = xw.tile([P, D // P, SW], BF16, tag="xwin")
                nc.sync.dma_start(xt, src[:, :, sw * SW:(sw + 1) * SW])
                emit(sw, xt)

        def emit_k(sw, xt):
            for fo in range(FG // P):
                ps = projp.tile([P, SW], F32, tag="pp")
                for ko in range(D // P):
                    nc.tensor.matmul(ps, lhsT=wk_sb[:, ko, fo * P:(fo + 1) * P],
                                     rhs=xt[:, ko, :],
                                     start=(ko == 0), stop=(ko == D // P - 1))
                sl = slice(sw * SW, (sw + 1) * SW)
                for half in range(2):
                    hh = fo * 2 + half
                    nc.vector.tensor_tensor(
                        KTp[half * D_K:(half + 1) * D_K, hh, sl],
                        ps[half * D_K:(half + 1) * D_K, :],
                        bk_sb[half * D_K:(half + 1) * D_K, fo:fo + 1]
                        .to_broadcast((D_K, SW)),
                        mybir.AluOpType.add)

        def emit_v(sw, xt):
            for st in range(SW // P):
                ps = projp.tile([P, FG], F32, tag="pp")
                for ko in range(D // P):
                    nc.tensor.matmul(ps, lhsT=xt[:, ko, st * P:(st + 1) * P],
                                     rhs=wv_sb[:, ko, :],
                                     start=(ko == 0), stop=(ko == D // P - 1))
                for hh in range(HPG):
                    nc.vector.tensor_tensor(
                        VA[:, sw * (SW // P) + st, hh, 0:D_K],
                        ps[:, hh * D_K:(hh + 1) * D_K],
                        bv_sb[:, hh * D_K:(hh + 1) * D_K],
                        mybir.AluOpType.add)

        def emit_q(sw, xt):
            for fo in range(FG // P):
                ps = projp.tile([P, SW], F32, tag="pp")
                for ko in range(D // P):
                    nc.tensor.matmul(ps, lhsT=wq_sb[:, ko, fo * P:(fo + 1) * P],
                                     rhs=xt[:, ko, :],
                                     start=(ko == 0), stop=(ko == D // P - 1))
                nc.vector.tensor_tensor(
                    QT[:, fo, sw * SW:(sw + 1) * SW], ps,
                    bq_sb[:, fo:fo + 1].to_broadcast((P, SW)),
                    mybir.AluOpType.add)

        KV = S // P  # 32 kv tiles

        def alloc_oacc(key):
            oacc = []
            for _c in range(QW // SW):
                ot_acc = oP.tile([D_K + 1, SW], F32, tag="oacc",
                                 name=f"oacc_{key}_{_c}")
                oacc.append(ot_acc)
            return oacc

        def _mm2(oacc, hh, et, kv):
            for c in range(QW // SW):
                nc.tensor.matmul(
                    oacc[c], lhsT=VA[:, kv, hh, :],
                    rhs=et[:, c * SW:(c + 1) * SW],
                    start=(kv == 0), stop=(kv == KV - 1))

        def emit_attn(qw, hh, oacc, kv_range, pend=None):
            # Software-pipelined PE stream: emit kv's scores matmuls before
            # the previous kv's PV matmuls so PE works while ScalarE computes
            # exp, instead of stalling behind the exp->mm2 dependency.
            for kv in kv_range:
                st = sS.tile([P, QW], F32, tag="st")
                for c in range(QW // SW):
                    nc.tensor.matmul(
                        st[:, c * SW:(c + 1) * SW],
                        lhsT=KTp[:, hh, kv * P:(kv + 1) * P],
                        rhs=QT[:, hh // 2, qw * QW + c * SW: qw * QW + (c + 1) * SW],
                        start=True, stop=True)
                et = sE.tile([P, QW], BF16, tag="et")
                nc.scalar.activation(et, st,
                                     mybir.ActivationFunctionType.Exp,
                                     scale=SCALE)
                if not swpipe:
                    _mm2(oacc, hh, et, kv)
                    continue
                if pend is not None:
                    _mm2(oacc, hh, *pend)
                pend = (et, kv)
            return pend

        def emit_norm(qw, hh, oacc):
            # normalize: divide rows 0..63 by denominator row 64.
            # Keep the gpsimd broadcast OFF the PSUM-release chain: copy the
            # unnormalized rows + reciprocal out first (DVE only), then
            # broadcast + multiply from SBUF.
            for c in range(QW // SW):
                ots = slice(qw * QW + c * SW, qw * QW + (c + 1) * SW)
                otdst = OT[(hh % 2) * D_K:(hh % 2 + 1) * D_K, hh // 2, ots]
                if norm == "none":
                    nc.vector.tensor_copy(otdst, oacc[c][0:D_K, :])
                    continue
                rc = misc.tile([1, SW], F32, tag="rc")
                nc.vector.reciprocal(rc, oacc[c][D_K:D_K + 1, :])
                if norm == "decoupled":
                    otu = misc.tile([D_K, SW], F32, tag="otu")
                    nc.vector.tensor_copy(otu, oacc[c][0:D_K, :])
                    rcb = misc.tile([D_K, SW], F32, tag="rcb")
                    nc.gpsimd.partition_broadcast(rcb, rc)
                    nc.vector.tensor_tensor(otdst, otu, rcb,
                                            mybir.AluOpType.mult)
                else:  # "chained": original form
                    rcb = misc.tile([D_K, SW], F32, tag="rcb")
                    nc.gpsimd.partition_broadcast(rcb, rc)
                    nc.vector.tensor_tensor(otdst, oacc[c][0:D_K, :], rcb,
                                            mybir.AluOpType.mult)

        for _rep in range(reps):
          if include_proj:
            # Q for the first attention q-window, then K/V windows
            # interleaved with head 0's attention kv-chunks so ScalarE (exp)
            # starts working during the projection phase.
            proj_windows(xqT, emit_q, windows=range(QW // SW))
            if include_attn:
                if interleave01:
                    oacc00 = alloc_oacc(f"h0_{_rep}")
                    pend = None
                    for w in range(S // SW):
                        proj_windows(xkT, emit_k, windows=[w])
                        proj_windows(xvT, emit_v, windows=[w])
                        pend = emit_attn(0, 0, oacc00,
                                         range(w * (SW // P),
                                               (w + 1) * (SW // P)),
                                         pend)
                    if pend is not None:
                        _mm2(oacc00, 0, *pend)
                    emit_norm(0, 0, oacc00)
                else:
                    proj_windows(xkT, emit_k)
                    proj_windows(xvT, emit_v)
                    oacc00 = alloc_oacc(f"h0_{_rep}")
                    pend = emit_attn(0, 0, oacc00, range(KV))
                    if pend is not None:
                        _mm2(oacc00, 0, *pend)
                    emit_norm(0, 0, oacc00)
            else:
                proj_windows(xkT, emit_k)
                proj_windows(xvT, emit_v)
          elif include_attn:
            oacc00 = alloc_oacc(f"h0_{_rep}")
            pend = emit_attn(0, 0, oacc00, range(KV))
            if pend is not None:
                _mm2(oacc00, 0, *pend)
            emit_norm(0, 0, oacc00)

          # ---- attention + output projection ----
          def emit_final(qt):
              for on in range(D // SW):
                  yp = projp.tile([P, SW], F32, tag="pp")
                  for fo in range(FG // P):
                      nc.tensor.matmul(
                          yp, lhsT=OT[:, fo, qt * P:(qt + 1) * P],
                          rhs=wo_sb[:, fo, on * SW:(on + 1) * SW],
                          start=(fo == 0), stop=(fo == FG // P - 1))
                  ys = yout.tile([P, SW], F32, tag="ys")
                  nc.vector.tensor_copy(ys, yp)
                  nc.sync.dma_start(y_r[qt, :, on * SW:(on + 1) * SW], ys)

          QT_PER_W = QW // P  # q-tiles per window
          for qw in range(S // QW):
            for hh in range(HPG if include_attn else 0):
                if not (qw == 0 and hh == 0):
                    oacc = alloc_oacc(f"{_rep}_{qw}_{hh}")
                    pend = emit_attn(qw, hh, oacc, range(KV))
                    if pend is not None:
                        _mm2(oacc, hh, *pend)
                    emit_norm(qw, hh, oacc)
                # spread the previous window's output projection and the
                # next window's Q projection across this window's heads so
                # PE-side filler work never starves ScalarE for long.
                if spread:
                    if include_final and qw > 0:
                        qt0 = (qw - 1) * QT_PER_W
                        emit_final(qt0 + 2 * hh)
                        emit_final(qt0 + 2 * hh + 1)
                    if include_proj and qw + 1 < S // QW and hh in (1, 3):
                        proj_windows(xqT, emit_q,
                                     windows=[(qw + 1) * (QW // SW) + hh // 2])
            if include_attn and not spread:
                if include_proj and qw + 1 < S // QW:
                    lo = (qw + 1) * (QW // SW)
                    proj_windows(xqT, emit_q, windows=range(lo, lo + QW // SW))
                if include_final:
                    for qt in range(qw * QT_PER_W, (qw + 1) * QT_PER_W):
                        emit_final(qt)
            if not include_attn:
                if include_proj and qw + 1 < S // QW:
                    lo = (qw + 1) * (QW // SW)
                    proj_windows(xqT, emit_q, windows=range(lo, lo + QW // SW))
                if include_final and qw > 0:
                    for qt in range((qw - 1) * QT_PER_W, qw * QT_PER_W):
                        emit_final(qt)
          if include_final and (spread or not include_attn):
            for qt in range((S // QW - 1) * QT_PER_W, S // P):
                emit_final(qt)

    nc.compile()
    return nc


def _dmajor(a2d, free):
    """[rows, 1024-like dim] bf16 -> [128, dim//128, free] with partition =
    inner index of the leading (transposed) dim."""
    d = a2d.shape[1]
    t = np.ascontiguousarray(a2d.astype(NPBF).T)          # [d, free]
    return np.ascontiguousarray(t.reshape(d // P, P, free).transpose(1, 0, 2))


_NC_CACHE = {}


def make_in_maps(query, key, value, Wq, bq, Wk, bk, Wv, bv, Wo, bo):
    query = np.asarray(query, np.float32)
    key = np.asarray(key, np.float32)
    value = np.asarray(value, np.float32)

    # per-batch transposed bf16 activations (shared by 4 cores each)
    xT = {b: {"q": _dmajor(query[b], S), "k": _dmajor(key[b], S),
              "v": _dmajor(value[b], S)} for b in range(B)}

    in_maps = []
    for c in range(N_CORES):
        b, hg = divmod(c, HG)
        fsl = slice(hg * FG, (hg + 1) * FG)
        in_maps.append({
            "xqT": xT[b]["q"], "xkT": xT[b]["k"], "xvT": xT[b]["v"],
            "wqT": _dmajor(np.asarray(Wq)[fsl], FG),
            "wkT": _dmajor(np.asarray(Wk)[fsl], FG),
            "wvT": _dmajor(np.asarray(Wv)[fsl], FG),
            "woT": _dmajor(np.asarray(Wo)[:, fsl], D),
            "bq2": np.ascontiguousarray(
                np.asarray(bq, np.float32)[fsl].reshape(FG // P, P).T),
            "bk2": np.ascontiguousarray(
                np.asarray(bk, np.float32)[fsl].reshape(FG // P, P).T),
            "bv1": np.asarray(bv, np.float32)[fsl].reshape(1, FG),
        })
    return in_maps


def kernel(query, key, value, Wq, bq, Wk, bk, Wv, bv, Wo, bo):
    if "nc" not in _NC_CACHE:
        _NC_CACHE["nc"] = build_nc()
    nc = _NC_CACHE["nc"]

    in_maps = make_in_maps(query, key, value, Wq, bq, Wk, bk, Wv, bv, Wo, bo)
    res = run_bass_kernel_spmd(nc, in_maps, core_ids=list(range(N_CORES)))

    out = np.zeros((B, S, D), np.float32)
    for c in range(N_CORES):
        out[c // HG] += res.results[c]["y"]
    out += np.asarray(bo, np.float32)
    return out
